# revision 4
# baseline (speedup 1.0000x reference)
"""Grok1-style MoE (E=8 experts, top-2, H=2048, I=4096, T=8192) on 8 trn2 NeuronCores.

Strategy: tensor parallelism over the intermediate dim I, host-side routing.
- Host computes the (tiny) router matmul + softcapped softmax + top-2, packs
  ALL routed (token, expert) columns into one expert-sorted column list of
  length Ctot = sum(counts) = T*TOPK.  Every core processes the SAME column
  list; core c holds I-rows [c*512, (c+1)*512) of every expert's w1/w3/w2.
  Per-core work is exactly Ctot/8 column-equivalents regardless of routing
  imbalance, and the program structure (expert section boundaries) is a
  compile-time constant shared by all cores -> clean SPMD.
- Device kernel per core, per expert section (bf16 matmuls, fp32 accumulate):
    hT[it]  = silu(w1sl.T @ xT) * (w3sl.T @ xT)   # it in 0..3 (512 I-rows)
    outT   += w2sl.T @ hT                          # [H, Ce] partial (bf16)
- Host sums the 8 bf16 partials in fp32 and scatter-adds probs-weighted
  columns into the full output.
"""

import os
import sys

for _p in ("/opt/trn_rl_repo", "/root/.axon_site/_ro/trn_rl_repo"):
    if os.path.isdir(_p) and _p not in sys.path:
        sys.path.insert(0, _p)

import numpy as np
import ml_dtypes

import concourse.bass as bass  # noqa: F401  (registers types)
import concourse.mybir as mybir
import concourse.tile as tile
from concourse import bacc
from concourse.bass_utils import run_bass_kernel_spmd

BF16 = mybir.dt.bfloat16
F32 = mybir.dt.float32
AF = mybir.ActivationFunctionType

E, TOPK, H, I = 8, 2, 2048, 4096
SOFTCAP = 30.0
KH = H // 128     # 16 k-tiles over H
ISL = I // 8      # 512 I-rows per core per expert
KL = ISL // 128   # 4 local k-tiles over the I slice
GROUP_MAX = 1152  # max token-columns resident per group (SBUF budget)

_prog_cache: dict = {}


def _chunks(C: int):
    """Split [0, C) into matmul-N chunks <=512; avoid a short tail chunk."""
    ws = []
    c = 0
    while c < C:
        w = min(512, C - c)
        ws.append(w)
        c += w
    if len(ws) >= 2 and ws[-1] < 256:
        tot = ws[-2] + ws[-1]
        a = (tot + 1) // 2
        ws[-2:] = [a, tot - a]
    out = []
    c = 0
    for w in ws:
        out.append((c, w))
        c += w
    return out


def _groups(C: int):
    """Pack chunks into SBUF-resident groups: (g0, gw, [(rel_off, w), ...])."""
    groups = []
    cur, cur_w = [], 0
    for off, w in _chunks(C):
        if cur and cur_w + w > GROUP_MAX:
            groups.append((cur[0][0], cur_w, [(o - cur[0][0], ww) for o, ww in cur]))
            cur, cur_w = [], 0
        cur.append((off, w))
        cur_w += w
    if cur:
        groups.append((cur[0][0], cur_w, [(o - cur[0][0], ww) for o, ww in cur]))
    return groups


def _build_program(counts):
    key = tuple(counts)
    if key in _prog_cache:
        return _prog_cache[key]

    Ctot = int(sum(counts))
    nc = bacc.Bacc(None, target_bir_lowering=False)

    xT_d = nc.declare_dram_parameter("xT", [128, KH, Ctot], BF16, isOutput=False)
    w1_d = nc.declare_dram_parameter("w1t", [E, KL, 128, KH, 128], BF16, isOutput=False)
    w3_d = nc.declare_dram_parameter("w3t", [E, KL, 128, KH, 128], BF16, isOutput=False)
    w2_d = nc.declare_dram_parameter("w2t", [E, 128, KH, KL, 128], BF16, isOutput=False)
    out_d = nc.declare_dram_parameter("outT", [KH, 128, Ctot], BF16, isOutput=True)

    with tile.TileContext(nc) as tc:
        with (
            tc.tile_pool(name="xg", bufs=1) as xp,
            tc.tile_pool(name="hT", bufs=2) as hp,
            tc.tile_pool(name="wstrip", bufs=2) as wp,
            tc.tile_pool(name="w2t", bufs=2) as w2p,
            tc.tile_pool(name="evac", bufs=3) as ep,
            tc.tile_pool(name="ps", bufs=2, space="PSUM") as psp,
            tc.tile_pool(name="pso", bufs=3, space="PSUM") as psop,
            tc.tile_pool(name="wu", bufs=1) as wup,
            tc.tile_pool(name="wups", bufs=1, space="PSUM") as wupsp,
        ):
            # Warm-up: ~5us of throwaway matmuls so the PE HAM clock-gate
            # reaches 8/8 while the first token/weight DMAs are in flight.
            wu_a = wup.tile([128, 512], BF16, tag="wua")
            nc.vector.memset(wu_a[:], 0.0)
            wu_ps = wupsp.tile([128, 512], F32, tag="wups")
            for _ in range(8):
                nc.tensor.matmul(wu_ps[:], wu_a[:, :128], wu_a[:], start=True, stop=True)

            first = True
            off_e = 0
            for e in range(E):
                Ce = int(counts[e])
                if Ce == 0:
                    continue
                w2t = None
                for gi, (g0, gw, chunks) in enumerate(_groups(Ce)):
                    a0 = off_e + g0
                    # Very first group: the opening matmul chain needs the
                    # it=0 weight strips before anything else.
                    pre_w = {}
                    if first:
                        w1s = wp.tile([128, KH, 128], BF16, tag="w1")
                        w3s = wp.tile([128, KH, 128], BF16, tag="w3")
                        nc.sync.dma_start(w1s[:], w1_d[e, 0])
                        nc.sync.dma_start(w3s[:], w3_d[e, 0])
                        pre_w[0] = (w1s, w3s)
                    # per-k x tiles so the first matmul chain only waits on
                    # 1/KH of the group's token load
                    xgk = []
                    for k in range(KH):
                        t = xp.tile([128, gw], BF16, tag=f"xg{k}")
                        nc.sync.dma_start(t[:], xT_d[:, k, a0 : a0 + gw])
                        xgk.append(t)
                    if first:
                        w1s = wp.tile([128, KH, 128], BF16, tag="w1")
                        w3s = wp.tile([128, KH, 128], BF16, tag="w3")
                        nc.sync.dma_start(w1s[:], w1_d[e, 1])
                        nc.sync.dma_start(w3s[:], w3_d[e, 1])
                        pre_w[1] = (w1s, w3s)
                        first = False
                    if gi == 0:
                        # w2 slice for this expert: resident for the whole
                        # section; queued after the stage-1 critical loads.
                        w2t = w2p.tile([128, KH, KL, 128], BF16, tag="w2")
                        nc.sync.dma_start(w2t[:], w2_d[e])
                    hT = hp.tile([128, KL, gw], BF16, tag="hT")
                    # ---- stage 1: hT[it] = silu(w1.T x) * (w3.T x) ----
                    for it in range(KL):
                        if it in pre_w:
                            w1s, w3s = pre_w[it]
                        else:
                            w1s = wp.tile([128, KH, 128], BF16, tag="w1")
                            w3s = wp.tile([128, KH, 128], BF16, tag="w3")
                            nc.sync.dma_start(w1s[:], w1_d[e, it])
                            nc.sync.dma_start(w3s[:], w3_d[e, it])
                        for c0, cw in chunks:
                            ps1 = psp.tile([128, cw], F32, tag="ps1")
                            ps3 = psp.tile([128, cw], F32, tag="ps3")
                            for k in range(KH):
                                nc.tensor.matmul(
                                    ps1[:], w1s[:, k, :], xgk[k][:, c0 : c0 + cw],
                                    start=(k == 0), stop=(k == KH - 1),
                                )
                                nc.tensor.matmul(
                                    ps3[:], w3s[:, k, :], xgk[k][:, c0 : c0 + cw],
                                    start=(k == 0), stop=(k == KH - 1),
                                )
                            st = ep.tile([128, cw], F32, tag="silu")
                            nc.scalar.activation(st[:], ps1[:], AF.Silu)
                            nc.vector.tensor_mul(hT[:, it, c0 : c0 + cw], st[:], ps3[:])
                    # ---- stage 2: outT[ht] += w2.T hT (partial, bf16) ----
                    for ht in range(KH):
                        for c0, cw in chunks:
                            pso = psop.tile([128, cw], F32, tag="pso")
                            for k in range(KL):
                                nc.tensor.matmul(
                                    pso[:], w2t[:, ht, k, :], hT[:, k, c0 : c0 + cw],
                                    start=(k == 0), stop=(k == KL - 1),
                                )
                            ot = ep.tile([128, cw], BF16, tag="ot")
                            nc.vector.tensor_copy(ot[:], pso[:])
                            nc.sync.dma_start(
                                out_d[ht, :, a0 + c0 : a0 + c0 + cw], ot[:]
                            )
                off_e += Ce
    nc.finalize()
    _prog_cache[key] = nc
    return nc


def _route(x: np.ndarray, w_gate: np.ndarray):
    """Replicates the reference router in fp32: softcapped softmax + top-2."""
    logits = x @ w_gate
    logits = (SOFTCAP * np.tanh(logits / SOFTCAP)).astype(np.float32)
    m = logits.max(axis=-1, keepdims=True)
    e = np.exp(logits - m)
    probs = e / e.sum(axis=-1, keepdims=True)
    idx = np.argsort(-probs, axis=-1, kind="stable")[:, :TOPK]
    return probs, idx


def _run(inputs, trace=False, trace_kwargs=None):
    hidden_states = np.asarray(inputs["hidden_states"], dtype=np.float32)
    w_gate = np.asarray(inputs["w_gate"], dtype=np.float32)
    w1 = np.asarray(inputs["w1"], dtype=np.float32)
    w3 = np.asarray(inputs["w3"], dtype=np.float32)
    w2 = np.asarray(inputs["w2"], dtype=np.float32)

    orig_shape = hidden_states.shape
    x = hidden_states.reshape(-1, H)
    T = x.shape[0]

    probs, idx = _route(x, w_gate)
    sel = np.zeros((T, E), dtype=bool)
    sel[np.arange(T), idx[:, 0]] = True
    sel[np.arange(T), idx[:, 1]] = True
    tok_idx = [np.nonzero(sel[:, e])[0] for e in range(E)]
    counts = [len(t) for t in tok_idx]
    offs = np.concatenate([[0], np.cumsum(counts)]).astype(np.int64)
    Ctot = int(offs[-1])

    nc = _build_program(counts)

    x_bf = x.astype(ml_dtypes.bfloat16)
    xg = np.empty((Ctot, H), dtype=ml_dtypes.bfloat16)
    for e in range(E):
        xg[offs[e] : offs[e + 1]] = x_bf[tok_idx[e]]
    # xT layout [128 p, KH k, Ctot c] with element [p,k,c] = xg[c, k*128+p]
    xT = np.ascontiguousarray(xg.T.reshape(KH, 128, Ctot).transpose(1, 0, 2))

    w1_bf = w1.astype(ml_dtypes.bfloat16)
    w3_bf = w3.astype(ml_dtypes.bfloat16)
    w2_bf = w2.astype(ml_dtypes.bfloat16)
    in_maps = []
    for c in range(8):
        sl = slice(c * ISL, (c + 1) * ISL)
        # [E, KL, 128(hsub), KH, 128(isub)]
        w1t = np.ascontiguousarray(
            w1_bf[:, :, sl].reshape(E, KH, 128, KL, 128).transpose(0, 3, 2, 1, 4)
        )
        w3t = np.ascontiguousarray(
            w3_bf[:, :, sl].reshape(E, KH, 128, KL, 128).transpose(0, 3, 2, 1, 4)
        )
        # [E, 128(isub), KH(ht), KL(k), 128(hsub)]
        w2t = np.ascontiguousarray(
            w2_bf[:, sl, :].reshape(E, KL, 128, KH, 128).transpose(0, 2, 3, 1, 4)
        )
        in_maps.append({"xT": xT, "w1t": w1t, "w3t": w3t, "w2t": w2t})

    res = run_bass_kernel_spmd(
        nc, in_maps, core_ids=list(range(8)), trace=trace,
        **(trace_kwargs or {}),
    )

    outT = np.zeros((H, Ctot), dtype=np.float32)
    for c in range(8):
        outT += res.results[c]["outT"].reshape(H, Ctot).astype(np.float32)

    out = np.zeros((T, H), dtype=np.float32)
    for e in range(E):
        wt = probs[tok_idx[e], e].astype(np.float32)
        out[tok_idx[e]] += outT[:, offs[e] : offs[e + 1]].T * wt[:, None]
    return out.reshape(orig_shape), res


def kernel(**inputs) -> np.ndarray:
    out, _ = _run(inputs, trace=False)
    return out


# revision 6
# speedup vs baseline: 1.0348x; 1.0348x over previous
"""Grok1-style MoE (E=8 experts, top-2, H=2048, I=4096, T=8192) on 8 trn2 NeuronCores.

Strategy: tensor parallelism over the intermediate dim I, host-side routing.
- Host computes the (tiny) router matmul + softcapped softmax + top-2, packs
  ALL routed (token, expert) columns into one expert-sorted column list of
  length Ctot = sum(counts) = T*TOPK.  Every core processes the SAME column
  list; core c holds I-rows [c*512, (c+1)*512) of every expert's w1/w3/w2.
  Per-core work is exactly Ctot/8 column-equivalents regardless of routing
  imbalance, and the program structure (expert section boundaries) is a
  compile-time constant shared by all cores -> clean SPMD.
- Device kernel per core, per expert section (bf16 matmuls, fp32 accumulate):
    hT[it]  = silu(w1sl.T @ xT) * (w3sl.T @ xT)   # it in 0..3 (512 I-rows)
    outT   += w2sl.T @ hT                          # [H, Ce] partial (bf16)
- Host sums the 8 bf16 partials in fp32 and scatter-adds probs-weighted
  columns into the full output.
"""

import os
import sys

for _p in ("/opt/trn_rl_repo", "/root/.axon_site/_ro/trn_rl_repo"):
    if os.path.isdir(_p) and _p not in sys.path:
        sys.path.insert(0, _p)

import numpy as np
import ml_dtypes

import concourse.bass as bass  # noqa: F401  (registers types)
import concourse.mybir as mybir
import concourse.tile as tile
from concourse import bacc
from concourse.bass_utils import run_bass_kernel_spmd

BF16 = mybir.dt.bfloat16
F32 = mybir.dt.float32
AF = mybir.ActivationFunctionType

E, TOPK, H, I = 8, 2, 2048, 4096
SOFTCAP = 30.0
KH = H // 128     # 16 k-tiles over H
ISL = I // 8      # 512 I-rows per core per expert
KL = ISL // 128   # 4 local k-tiles over the I slice
GROUP_MAX = 1152  # max token-columns resident per group (SBUF budget)

_prog_cache: dict = {}


def _chunks(C: int):
    """Split [0, C) into matmul-N chunks <=512; avoid a short tail chunk."""
    ws = []
    c = 0
    while c < C:
        w = min(512, C - c)
        ws.append(w)
        c += w
    if len(ws) >= 2 and ws[-1] < 256:
        tot = ws[-2] + ws[-1]
        a = (tot + 1) // 2
        ws[-2:] = [a, tot - a]
    out = []
    c = 0
    for w in ws:
        out.append((c, w))
        c += w
    return out


def _groups(C: int):
    """Pack chunks into SBUF-resident groups: (g0, gw, [(rel_off, w), ...])."""
    groups = []
    cur, cur_w = [], 0
    for off, w in _chunks(C):
        if cur and cur_w + w > GROUP_MAX:
            groups.append((cur[0][0], cur_w, [(o - cur[0][0], ww) for o, ww in cur]))
            cur, cur_w = [], 0
        cur.append((off, w))
        cur_w += w
    if cur:
        groups.append((cur[0][0], cur_w, [(o - cur[0][0], ww) for o, ww in cur]))
    return groups


def _build_program(counts):
    key = tuple(counts)
    if key in _prog_cache:
        return _prog_cache[key]

    Ctot = int(sum(counts))
    nc = bacc.Bacc(None, target_bir_lowering=False)

    xT_d = nc.declare_dram_parameter("xT", [128, KH, Ctot], BF16, isOutput=False)
    w1_d = nc.declare_dram_parameter("w1t", [E, KL, 128, KH, 128], BF16, isOutput=False)
    w3_d = nc.declare_dram_parameter("w3t", [E, KL, 128, KH, 128], BF16, isOutput=False)
    w2_d = nc.declare_dram_parameter("w2t", [E, 128, KH, KL, 128], BF16, isOutput=False)
    out_d = nc.declare_dram_parameter("outT", [KH, 128, Ctot], BF16, isOutput=True)

    with tile.TileContext(nc) as tc:
        with (
            tc.tile_pool(name="xg", bufs=1) as xp,
            tc.tile_pool(name="hT", bufs=2) as hp,
            tc.tile_pool(name="wstrip", bufs=2) as wp,
            tc.tile_pool(name="w2t", bufs=2) as w2p,
            tc.tile_pool(name="evac", bufs=4) as ep,
            tc.tile_pool(name="ps", bufs=2, space="PSUM") as psp,
            tc.tile_pool(name="pso", bufs=3, space="PSUM") as psop,
            tc.tile_pool(name="wu", bufs=1) as wup,
            tc.tile_pool(name="wups", bufs=1, space="PSUM") as wupsp,
        ):
            # Warm-up: ~5us of throwaway matmuls so the PE HAM clock-gate
            # reaches 8/8 while the first token/weight DMAs are in flight.
            wu_a = wup.tile([128, 512], BF16, tag="wua")
            nc.vector.memset(wu_a[:], 0.0)
            wu_ps = wupsp.tile([128, 512], F32, tag="wups")
            for _ in range(8):
                nc.tensor.matmul(wu_ps[:], wu_a[:, :128], wu_a[:], start=True, stop=True)

            first = True
            off_e = 0
            for e in range(E):
                Ce = int(counts[e])
                if Ce == 0:
                    continue
                w2t = None
                for gi, (g0, gw, chunks) in enumerate(_groups(Ce)):
                    a0 = off_e + g0
                    # Very first group: the opening matmul chain needs the
                    # it=0 weight strips before anything else.
                    pre_w = {}
                    if first:
                        w1s = wp.tile([128, KH, 128], BF16, tag="w1")
                        w3s = wp.tile([128, KH, 128], BF16, tag="w3")
                        nc.sync.dma_start(w1s[:], w1_d[e, 0])
                        nc.sync.dma_start(w3s[:], w3_d[e, 0])
                        pre_w[0] = (w1s, w3s)
                    # per-k x tiles so the first matmul chain only waits on
                    # 1/KH of the group's token load
                    xgk = []
                    for k in range(KH):
                        t = xp.tile([128, gw], BF16, tag=f"xg{k}")
                        nc.sync.dma_start(t[:], xT_d[:, k, a0 : a0 + gw])
                        xgk.append(t)
                    if first:
                        w1s = wp.tile([128, KH, 128], BF16, tag="w1")
                        w3s = wp.tile([128, KH, 128], BF16, tag="w3")
                        nc.sync.dma_start(w1s[:], w1_d[e, 1])
                        nc.sync.dma_start(w3s[:], w3_d[e, 1])
                        pre_w[1] = (w1s, w3s)
                        first = False
                    if gi == 0:
                        # w2 slice for this expert: resident for the whole
                        # section; queued after the stage-1 critical loads.
                        w2t = w2p.tile([128, KH, KL, 128], BF16, tag="w2")
                        nc.sync.dma_start(w2t[:], w2_d[e])
                    hT = hp.tile([128, KL, gw], BF16, tag="hT")
                    # ---- stage 1: hT[it] = silu(w1.T x) * (w3.T x) ----
                    for it in range(KL):
                        if it in pre_w:
                            w1s, w3s = pre_w[it]
                        else:
                            w1s = wp.tile([128, KH, 128], BF16, tag="w1")
                            w3s = wp.tile([128, KH, 128], BF16, tag="w3")
                            nc.sync.dma_start(w1s[:], w1_d[e, it])
                            nc.sync.dma_start(w3s[:], w3_d[e, it])
                        for c0, cw in chunks:
                            ps1 = psp.tile([128, cw], F32, tag="ps1")
                            ps3 = psp.tile([128, cw], F32, tag="ps3")
                            for k in range(KH):
                                nc.tensor.matmul(
                                    ps1[:], w1s[:, k, :], xgk[k][:, c0 : c0 + cw],
                                    start=(k == 0), stop=(k == KH - 1),
                                )
                                nc.tensor.matmul(
                                    ps3[:], w3s[:, k, :], xgk[k][:, c0 : c0 + cw],
                                    start=(k == 0), stop=(k == KH - 1),
                                )
                            st = ep.tile([128, cw], F32, tag="silu")
                            nc.scalar.activation(st[:], ps1[:], AF.Silu)
                            nc.vector.tensor_mul(hT[:, it, c0 : c0 + cw], st[:], ps3[:])
                    # ---- stage 2: outT[ht] += w2.T hT (partial, bf16) ----
                    for ht in range(KH):
                        for c0, cw in chunks:
                            pso = psop.tile([128, cw], F32, tag="pso")
                            for k in range(KL):
                                nc.tensor.matmul(
                                    pso[:], w2t[:, ht, k, :], hT[:, k, c0 : c0 + cw],
                                    start=(k == 0), stop=(k == KL - 1),
                                )
                            ot = ep.tile([128, cw], BF16, tag="ot")
                            nc.vector.tensor_copy(ot[:], pso[:])
                            # out DMAs ride the (otherwise idle) GpSimd queue
                            # so input prefetch on the sync queue never waits
                            # behind an output burst.
                            nc.gpsimd.dma_start(
                                out_d[ht, :, a0 + c0 : a0 + c0 + cw], ot[:]
                            )
                off_e += Ce
    nc.finalize()
    _prog_cache[key] = nc
    return nc


def _route(x: np.ndarray, w_gate: np.ndarray):
    """Replicates the reference router in fp32: softcapped softmax + top-2."""
    logits = x @ w_gate
    logits = (SOFTCAP * np.tanh(logits / SOFTCAP)).astype(np.float32)
    m = logits.max(axis=-1, keepdims=True)
    e = np.exp(logits - m)
    probs = e / e.sum(axis=-1, keepdims=True)
    idx = np.argsort(-probs, axis=-1, kind="stable")[:, :TOPK]
    return probs, idx


def _run(inputs, trace=False, trace_kwargs=None):
    hidden_states = np.asarray(inputs["hidden_states"], dtype=np.float32)
    w_gate = np.asarray(inputs["w_gate"], dtype=np.float32)
    w1 = np.asarray(inputs["w1"], dtype=np.float32)
    w3 = np.asarray(inputs["w3"], dtype=np.float32)
    w2 = np.asarray(inputs["w2"], dtype=np.float32)

    orig_shape = hidden_states.shape
    x = hidden_states.reshape(-1, H)
    T = x.shape[0]

    probs, idx = _route(x, w_gate)
    sel = np.zeros((T, E), dtype=bool)
    sel[np.arange(T), idx[:, 0]] = True
    sel[np.arange(T), idx[:, 1]] = True
    tok_idx = [np.nonzero(sel[:, e])[0] for e in range(E)]
    counts = [len(t) for t in tok_idx]
    offs = np.concatenate([[0], np.cumsum(counts)]).astype(np.int64)
    Ctot = int(offs[-1])

    nc = _build_program(counts)

    x_bf = x.astype(ml_dtypes.bfloat16)
    xg = np.empty((Ctot, H), dtype=ml_dtypes.bfloat16)
    for e in range(E):
        xg[offs[e] : offs[e + 1]] = x_bf[tok_idx[e]]
    # xT layout [128 p, KH k, Ctot c] with element [p,k,c] = xg[c, k*128+p]
    xT = np.ascontiguousarray(xg.T.reshape(KH, 128, Ctot).transpose(1, 0, 2))

    w1_bf = w1.astype(ml_dtypes.bfloat16)
    w3_bf = w3.astype(ml_dtypes.bfloat16)
    w2_bf = w2.astype(ml_dtypes.bfloat16)
    in_maps = []
    for c in range(8):
        sl = slice(c * ISL, (c + 1) * ISL)
        # [E, KL, 128(hsub), KH, 128(isub)]
        w1t = np.ascontiguousarray(
            w1_bf[:, :, sl].reshape(E, KH, 128, KL, 128).transpose(0, 3, 2, 1, 4)
        )
        w3t = np.ascontiguousarray(
            w3_bf[:, :, sl].reshape(E, KH, 128, KL, 128).transpose(0, 3, 2, 1, 4)
        )
        # [E, 128(isub), KH(ht), KL(k), 128(hsub)]
        w2t = np.ascontiguousarray(
            w2_bf[:, sl, :].reshape(E, KL, 128, KH, 128).transpose(0, 2, 3, 1, 4)
        )
        in_maps.append({"xT": xT, "w1t": w1t, "w3t": w3t, "w2t": w2t})

    res = run_bass_kernel_spmd(
        nc, in_maps, core_ids=list(range(8)), trace=trace,
        **(trace_kwargs or {}),
    )

    outT = np.zeros((H, Ctot), dtype=np.float32)
    for c in range(8):
        outT += res.results[c]["outT"].reshape(H, Ctot).astype(np.float32)

    out = np.zeros((T, H), dtype=np.float32)
    for e in range(E):
        wt = probs[tok_idx[e], e].astype(np.float32)
        out[tok_idx[e]] += outT[:, offs[e] : offs[e + 1]].T * wt[:, None]
    return out.reshape(orig_shape), res


def kernel(**inputs) -> np.ndarray:
    out, _ = _run(inputs, trace=False)
    return out


# revision 7
# speedup vs baseline: 1.1362x; 1.0980x over previous
"""Grok1-style MoE (E=8 experts, top-2, H=2048, I=4096, T=8192) on 8 trn2 NeuronCores.

Strategy: tensor parallelism over the intermediate dim I, host-side routing.
- Host computes the (tiny) router matmul + softcapped softmax + top-2, packs
  ALL routed (token, expert) columns into one expert-sorted column list of
  length Ctot = sum(counts) = T*TOPK.  Every core processes the SAME column
  list; core c holds I-rows [c*512, (c+1)*512) of every expert's w1/w3/w2.
  Per-core work is exactly Ctot/8 column-equivalents regardless of routing
  imbalance, and the program structure (expert section boundaries) is a
  compile-time constant shared by all cores -> clean SPMD.
- Device kernel per core, per expert section (bf16 matmuls, fp32 accumulate):
    hT[it]  = silu(w1sl.T @ xT) * (w3sl.T @ xT)   # it in 0..3 (512 I-rows)
    outT   += w2sl.T @ hT                          # [H, Ce] partial (bf16)
- Host sums the 8 bf16 partials in fp32 and scatter-adds probs-weighted
  columns into the full output.
"""

import os
import sys

for _p in ("/opt/trn_rl_repo", "/root/.axon_site/_ro/trn_rl_repo"):
    if os.path.isdir(_p) and _p not in sys.path:
        sys.path.insert(0, _p)

import numpy as np
import ml_dtypes

import concourse.bass as bass  # noqa: F401  (registers types)
import concourse.mybir as mybir
import concourse.tile as tile
from concourse import bacc
from concourse.bass_utils import run_bass_kernel_spmd

BF16 = mybir.dt.bfloat16
F32 = mybir.dt.float32
AF = mybir.ActivationFunctionType

E, TOPK, H, I = 8, 2, 2048, 4096
SOFTCAP = 30.0
KH = H // 128     # 16 k-tiles over H
ISL = I // 8      # 512 I-rows per core per expert
KL = ISL // 128   # 4 local k-tiles over the I slice
GROUP_MAX = 1152  # max token-columns resident per group (SBUF budget)

_prog_cache: dict = {}


def _chunks(C: int):
    """Split [0, C) into matmul-N chunks <=512; avoid a short tail chunk."""
    ws = []
    c = 0
    while c < C:
        w = min(512, C - c)
        ws.append(w)
        c += w
    if len(ws) >= 2 and ws[-1] < 256:
        tot = ws[-2] + ws[-1]
        a = (tot + 1) // 2
        ws[-2:] = [a, tot - a]
    out = []
    c = 0
    for w in ws:
        out.append((c, w))
        c += w
    return out


def _groups(C: int):
    """Pack chunks into SBUF-resident groups: (g0, gw, [(rel_off, w), ...])."""
    groups = []
    cur, cur_w = [], 0
    for off, w in _chunks(C):
        if cur and cur_w + w > GROUP_MAX:
            groups.append((cur[0][0], cur_w, [(o - cur[0][0], ww) for o, ww in cur]))
            cur, cur_w = [], 0
        cur.append((off, w))
        cur_w += w
    if cur:
        groups.append((cur[0][0], cur_w, [(o - cur[0][0], ww) for o, ww in cur]))
    return groups


def _build_program(counts):
    key = tuple(counts)
    if key in _prog_cache:
        return _prog_cache[key]

    Ctot = int(sum(counts))
    nc = bacc.Bacc(None, target_bir_lowering=False)

    xT_d = nc.declare_dram_parameter("xT", [128, KH, Ctot], BF16, isOutput=False)
    w1_d = nc.declare_dram_parameter("w1t", [E, KL, 128, KH, 128], BF16, isOutput=False)
    w3_d = nc.declare_dram_parameter("w3t", [E, KL, 128, KH, 128], BF16, isOutput=False)
    w2_d = nc.declare_dram_parameter("w2t", [E, 128, KH, KL, 128], BF16, isOutput=False)
    out_d = nc.declare_dram_parameter("outT", [KH, 128, Ctot], BF16, isOutput=True)

    with tile.TileContext(nc) as tc:
        with (
            tc.tile_pool(name="xg", bufs=1) as xp,
            tc.tile_pool(name="hT", bufs=2) as hp,
            tc.tile_pool(name="wstrip", bufs=2) as wp,
            tc.tile_pool(name="w2t", bufs=2) as w2p,
            tc.tile_pool(name="evac", bufs=4) as ep,
            tc.tile_pool(name="ps", bufs=2, space="PSUM") as psp,
            tc.tile_pool(name="pso", bufs=3, space="PSUM") as psop,
            tc.tile_pool(name="wu", bufs=1) as wup,
            tc.tile_pool(name="wups", bufs=1, space="PSUM") as wupsp,
        ):
            # Warm-up: ~5us of throwaway matmuls so the PE HAM clock-gate
            # reaches 8/8 while the first token/weight DMAs are in flight.
            wu_a = wup.tile([128, 512], BF16, tag="wua")
            nc.vector.memset(wu_a[:], 0.0)
            wu_ps = wupsp.tile([128, 512], F32, tag="wups")
            for _ in range(8):
                nc.tensor.matmul(wu_ps[:], wu_a[:, :128], wu_a[:], start=True, stop=True)

            first = True
            off_e = 0
            for e in range(E):
                Ce = int(counts[e])
                if Ce == 0:
                    continue
                w2t = None
                for gi, (g0, gw, chunks) in enumerate(_groups(Ce)):
                    a0 = off_e + g0
                    # Very first group: the opening matmul chain needs the
                    # it=0 weight strips before anything else.
                    pre_w = {}
                    if first:
                        w1s = wp.tile([128, KH, 128], BF16, tag="w1")
                        w3s = wp.tile([128, KH, 128], BF16, tag="w3")
                        nc.sync.dma_start(w1s[:], w1_d[e, 0])
                        nc.sync.dma_start(w3s[:], w3_d[e, 0])
                        pre_w[0] = (w1s, w3s)
                    # per-k x tiles so the first matmul chain only waits on
                    # 1/KH of the group's token load
                    xgk = []
                    for k in range(KH):
                        t = xp.tile([128, gw], BF16, tag=f"xg{k}")
                        nc.sync.dma_start(t[:], xT_d[:, k, a0 : a0 + gw])
                        xgk.append(t)
                    if first:
                        w1s = wp.tile([128, KH, 128], BF16, tag="w1")
                        w3s = wp.tile([128, KH, 128], BF16, tag="w3")
                        nc.sync.dma_start(w1s[:], w1_d[e, 1])
                        nc.sync.dma_start(w3s[:], w3_d[e, 1])
                        pre_w[1] = (w1s, w3s)
                        first = False
                    if gi == 0:
                        # w2 slice for this expert: resident for the whole
                        # section; queued after the stage-1 critical loads.
                        w2t = w2p.tile([128, KH, KL, 128], BF16, tag="w2")
                        nc.sync.dma_start(w2t[:], w2_d[e])
                    hT = hp.tile([128, KL, gw], BF16, tag="hT")
                    # ---- stage 1: hT[it] = silu(w1.T x) * (w3.T x) ----
                    for it in range(KL):
                        if it in pre_w:
                            w1s, w3s = pre_w[it]
                        else:
                            w1s = wp.tile([128, KH, 128], BF16, tag="w1")
                            w3s = wp.tile([128, KH, 128], BF16, tag="w3")
                            nc.sync.dma_start(w1s[:], w1_d[e, it])
                            nc.sync.dma_start(w3s[:], w3_d[e, it])
                        for c0, cw in chunks:
                            ps1 = psp.tile([128, cw], F32, tag="ps1")
                            ps3 = psp.tile([128, cw], F32, tag="ps3")
                            for k in range(KH):
                                nc.tensor.matmul(
                                    ps1[:], w1s[:, k, :], xgk[k][:, c0 : c0 + cw],
                                    start=(k == 0), stop=(k == KH - 1),
                                )
                                nc.tensor.matmul(
                                    ps3[:], w3s[:, k, :], xgk[k][:, c0 : c0 + cw],
                                    start=(k == 0), stop=(k == KH - 1),
                                )
                            st = ep.tile([128, cw], F32, tag="silu")
                            nc.scalar.activation(st[:], ps1[:], AF.Silu)
                            nc.vector.tensor_mul(hT[:, it, c0 : c0 + cw], st[:], ps3[:])
                    # ---- stage 2: outT[ht] += w2.T hT (partial, bf16) ----
                    for ht in range(KH):
                        ot = ep.tile([128, gw], BF16, tag="ot")
                        for c0, cw in chunks:
                            pso = psop.tile([128, cw], F32, tag="pso")
                            for k in range(KL):
                                nc.tensor.matmul(
                                    pso[:], w2t[:, ht, k, :], hT[:, k, c0 : c0 + cw],
                                    start=(k == 0), stop=(k == KL - 1),
                                )
                            nc.vector.tensor_copy(ot[:, c0 : c0 + cw], pso[:])
                        # One coalesced out DMA per (ht, group), on the
                        # (otherwise idle) GpSimd queue so input prefetch on
                        # the sync queue never waits behind an output burst.
                        nc.gpsimd.dma_start(out_d[ht, :, a0 : a0 + gw], ot[:])
                off_e += Ce
    nc.finalize()
    _prog_cache[key] = nc
    return nc


def _route(x: np.ndarray, w_gate: np.ndarray):
    """Replicates the reference router in fp32: softcapped softmax + top-2."""
    logits = x @ w_gate
    logits = (SOFTCAP * np.tanh(logits / SOFTCAP)).astype(np.float32)
    m = logits.max(axis=-1, keepdims=True)
    e = np.exp(logits - m)
    probs = e / e.sum(axis=-1, keepdims=True)
    idx = np.argsort(-probs, axis=-1, kind="stable")[:, :TOPK]
    return probs, idx


def _run(inputs, trace=False, trace_kwargs=None):
    hidden_states = np.asarray(inputs["hidden_states"], dtype=np.float32)
    w_gate = np.asarray(inputs["w_gate"], dtype=np.float32)
    w1 = np.asarray(inputs["w1"], dtype=np.float32)
    w3 = np.asarray(inputs["w3"], dtype=np.float32)
    w2 = np.asarray(inputs["w2"], dtype=np.float32)

    orig_shape = hidden_states.shape
    x = hidden_states.reshape(-1, H)
    T = x.shape[0]

    probs, idx = _route(x, w_gate)
    sel = np.zeros((T, E), dtype=bool)
    sel[np.arange(T), idx[:, 0]] = True
    sel[np.arange(T), idx[:, 1]] = True
    tok_idx = [np.nonzero(sel[:, e])[0] for e in range(E)]
    counts = [len(t) for t in tok_idx]
    offs = np.concatenate([[0], np.cumsum(counts)]).astype(np.int64)
    Ctot = int(offs[-1])

    nc = _build_program(counts)

    x_bf = x.astype(ml_dtypes.bfloat16)
    xg = np.empty((Ctot, H), dtype=ml_dtypes.bfloat16)
    for e in range(E):
        xg[offs[e] : offs[e + 1]] = x_bf[tok_idx[e]]
    # xT layout [128 p, KH k, Ctot c] with element [p,k,c] = xg[c, k*128+p]
    xT = np.ascontiguousarray(xg.T.reshape(KH, 128, Ctot).transpose(1, 0, 2))

    w1_bf = w1.astype(ml_dtypes.bfloat16)
    w3_bf = w3.astype(ml_dtypes.bfloat16)
    w2_bf = w2.astype(ml_dtypes.bfloat16)
    in_maps = []
    for c in range(8):
        sl = slice(c * ISL, (c + 1) * ISL)
        # [E, KL, 128(hsub), KH, 128(isub)]
        w1t = np.ascontiguousarray(
            w1_bf[:, :, sl].reshape(E, KH, 128, KL, 128).transpose(0, 3, 2, 1, 4)
        )
        w3t = np.ascontiguousarray(
            w3_bf[:, :, sl].reshape(E, KH, 128, KL, 128).transpose(0, 3, 2, 1, 4)
        )
        # [E, 128(isub), KH(ht), KL(k), 128(hsub)]
        w2t = np.ascontiguousarray(
            w2_bf[:, sl, :].reshape(E, KL, 128, KH, 128).transpose(0, 2, 3, 1, 4)
        )
        in_maps.append({"xT": xT, "w1t": w1t, "w3t": w3t, "w2t": w2t})

    res = run_bass_kernel_spmd(
        nc, in_maps, core_ids=list(range(8)), trace=trace,
        **(trace_kwargs or {}),
    )

    outT = np.zeros((H, Ctot), dtype=np.float32)
    for c in range(8):
        outT += res.results[c]["outT"].reshape(H, Ctot).astype(np.float32)

    out = np.zeros((T, H), dtype=np.float32)
    for e in range(E):
        wt = probs[tok_idx[e], e].astype(np.float32)
        out[tok_idx[e]] += outT[:, offs[e] : offs[e + 1]].T * wt[:, None]
    return out.reshape(orig_shape), res


def kernel(**inputs) -> np.ndarray:
    out, _ = _run(inputs, trace=False)
    return out


# revision 8
# speedup vs baseline: 1.1681x; 1.0280x over previous
"""Grok1-style MoE (E=8 experts, top-2, H=2048, I=4096, T=8192) on 8 trn2 NeuronCores.

Strategy: tensor parallelism over the intermediate dim I, host-side routing.
- Host computes the (tiny) router matmul + softcapped softmax + top-2, packs
  ALL routed (token, expert) columns into one expert-sorted column list of
  length Ctot = sum(counts) = T*TOPK.  Every core processes the SAME column
  list; core c holds I-rows [c*512, (c+1)*512) of every expert's w1/w3/w2.
  Per-core work is exactly Ctot/8 column-equivalents regardless of routing
  imbalance, and the program structure (expert section boundaries) is a
  compile-time constant shared by all cores -> clean SPMD.
- Device kernel per core, per expert section (bf16 matmuls, fp32 accumulate):
    hT[it]  = silu(w1sl.T @ xT) * (w3sl.T @ xT)   # it in 0..3 (512 I-rows)
    outT   += w2sl.T @ hT                          # [H, Ce] partial (bf16)
- Host sums the 8 bf16 partials in fp32 and scatter-adds probs-weighted
  columns into the full output.
"""

import os
import sys

for _p in ("/opt/trn_rl_repo", "/root/.axon_site/_ro/trn_rl_repo"):
    if os.path.isdir(_p) and _p not in sys.path:
        sys.path.insert(0, _p)

import numpy as np
import ml_dtypes

import concourse.bass as bass  # noqa: F401  (registers types)
import concourse.mybir as mybir
import concourse.tile as tile
from concourse import bacc
from concourse.bass_utils import run_bass_kernel_spmd

BF16 = mybir.dt.bfloat16
F32 = mybir.dt.float32
AF = mybir.ActivationFunctionType

E, TOPK, H, I = 8, 2, 2048, 4096
SOFTCAP = 30.0
KH = H // 128     # 16 k-tiles over H
ISL = I // 8      # 512 I-rows per core per expert
KL = ISL // 128   # 4 local k-tiles over the I slice
GROUP_MAX = 1152  # max token-columns resident per group (SBUF budget)

_prog_cache: dict = {}


def _chunks(C: int):
    """Split [0, C) into matmul-N chunks <=512; avoid a short tail chunk."""
    ws = []
    c = 0
    while c < C:
        w = min(512, C - c)
        ws.append(w)
        c += w
    if len(ws) >= 2 and ws[-1] < 256:
        tot = ws[-2] + ws[-1]
        a = (tot + 1) // 2
        ws[-2:] = [a, tot - a]
    out = []
    c = 0
    for w in ws:
        out.append((c, w))
        c += w
    return out


def _groups(C: int):
    """Pack chunks into SBUF-resident groups: (g0, gw, [(rel_off, w), ...])."""
    groups = []
    cur, cur_w = [], 0
    for off, w in _chunks(C):
        if cur and cur_w + w > GROUP_MAX:
            groups.append((cur[0][0], cur_w, [(o - cur[0][0], ww) for o, ww in cur]))
            cur, cur_w = [], 0
        cur.append((off, w))
        cur_w += w
    if cur:
        groups.append((cur[0][0], cur_w, [(o - cur[0][0], ww) for o, ww in cur]))
    return groups


def _build_program(counts):
    key = tuple(counts)
    if key in _prog_cache:
        return _prog_cache[key]

    Ctot = int(sum(counts))
    nc = bacc.Bacc(None, target_bir_lowering=False)

    xT_d = nc.declare_dram_parameter("xT", [128, KH, Ctot], BF16, isOutput=False)
    w1_d = nc.declare_dram_parameter("w1t", [E, KL, 128, KH, 128], BF16, isOutput=False)
    w3_d = nc.declare_dram_parameter("w3t", [E, KL, 128, KH, 128], BF16, isOutput=False)
    w2_d = nc.declare_dram_parameter("w2t", [E, 128, KH, KL, 128], BF16, isOutput=False)
    out_d = nc.declare_dram_parameter("outT", [KH, 128, Ctot], BF16, isOutput=True)

    with tile.TileContext(nc) as tc:
        with (
            tc.tile_pool(name="xg", bufs=2) as xp,
            tc.tile_pool(name="hT", bufs=2) as hp,
            tc.tile_pool(name="wstrip", bufs=2) as wp,
            tc.tile_pool(name="w2t", bufs=2) as w2p,
            tc.tile_pool(name="evac", bufs=8) as ep,
            tc.tile_pool(name="ps", bufs=2, space="PSUM") as psp,
            tc.tile_pool(name="pso", bufs=3, space="PSUM") as psop,
            tc.tile_pool(name="wu", bufs=1) as wup,
            tc.tile_pool(name="wups", bufs=1, space="PSUM") as wupsp,
        ):
            # Warm-up: ~5us of throwaway matmuls so the PE HAM clock-gate
            # reaches 8/8 while the first token/weight DMAs are in flight.
            wu_a = wup.tile([128, 512], BF16, tag="wua")
            nc.vector.memset(wu_a[:], 0.0)
            wu_ps = wupsp.tile([128, 512], F32, tag="wups")
            for _ in range(8):
                nc.tensor.matmul(wu_ps[:], wu_a[:, :128], wu_a[:], start=True, stop=True)

            first = True
            off_e = 0
            for e in range(E):
                Ce = int(counts[e])
                if Ce == 0:
                    continue
                w2t = None
                for gi, (g0, gw, chunks) in enumerate(_groups(Ce)):
                    a0 = off_e + g0
                    # Very first group: the opening matmul chain needs the
                    # it=0 weight strips before anything else.
                    pre_w = {}
                    if first:
                        w1s = wp.tile([128, KH, 128], BF16, tag="w1")
                        w3s = wp.tile([128, KH, 128], BF16, tag="w3")
                        nc.sync.dma_start(w1s[:], w1_d[e, 0])
                        nc.sync.dma_start(w3s[:], w3_d[e, 0])
                        pre_w[0] = (w1s, w3s)
                    # per-k x tiles so the first matmul chain only waits on
                    # 1/KH of the group's token load
                    xgk = []
                    for k in range(KH):
                        t = xp.tile([128, gw], BF16, tag=f"xg{k}")
                        nc.sync.dma_start(t[:], xT_d[:, k, a0 : a0 + gw])
                        xgk.append(t)
                    if first:
                        w1s = wp.tile([128, KH, 128], BF16, tag="w1")
                        w3s = wp.tile([128, KH, 128], BF16, tag="w3")
                        nc.sync.dma_start(w1s[:], w1_d[e, 1])
                        nc.sync.dma_start(w3s[:], w3_d[e, 1])
                        pre_w[1] = (w1s, w3s)
                        first = False
                    if gi == 0:
                        # w2 slice for this expert: resident for the whole
                        # section; queued after the stage-1 critical loads.
                        w2t = w2p.tile([128, KH, KL, 128], BF16, tag="w2")
                        nc.sync.dma_start(w2t[:], w2_d[e])
                    hT = hp.tile([128, KL, gw], BF16, tag="hT")
                    # ---- stage 1: hT[it] = silu(w1.T x) * (w3.T x) ----
                    for it in range(KL):
                        if it in pre_w:
                            w1s, w3s = pre_w[it]
                        else:
                            w1s = wp.tile([128, KH, 128], BF16, tag="w1")
                            w3s = wp.tile([128, KH, 128], BF16, tag="w3")
                            nc.sync.dma_start(w1s[:], w1_d[e, it])
                            nc.sync.dma_start(w3s[:], w3_d[e, it])
                        for c0, cw in chunks:
                            ps1 = psp.tile([128, cw], F32, tag="ps1")
                            ps3 = psp.tile([128, cw], F32, tag="ps3")
                            for k in range(KH):
                                nc.tensor.matmul(
                                    ps1[:], w1s[:, k, :], xgk[k][:, c0 : c0 + cw],
                                    start=(k == 0), stop=(k == KH - 1),
                                )
                                nc.tensor.matmul(
                                    ps3[:], w3s[:, k, :], xgk[k][:, c0 : c0 + cw],
                                    start=(k == 0), stop=(k == KH - 1),
                                )
                            st = ep.tile([128, cw], F32, tag="silu")
                            nc.scalar.activation(st[:], ps1[:], AF.Silu)
                            nc.vector.tensor_mul(hT[:, it, c0 : c0 + cw], st[:], ps3[:])
                    # ---- stage 2: outT[ht] += w2.T hT (partial, bf16) ----
                    for ht in range(KH):
                        ot = ep.tile([128, gw], BF16, tag="ot")
                        for c0, cw in chunks:
                            pso = psop.tile([128, cw], F32, tag="pso")
                            for k in range(KL):
                                nc.tensor.matmul(
                                    pso[:], w2t[:, ht, k, :], hT[:, k, c0 : c0 + cw],
                                    start=(k == 0), stop=(k == KL - 1),
                                )
                            nc.vector.tensor_copy(ot[:, c0 : c0 + cw], pso[:])
                        # One coalesced out DMA per (ht, group), on the
                        # (otherwise idle) GpSimd queue so input prefetch on
                        # the sync queue never waits behind an output burst.
                        nc.gpsimd.dma_start(out_d[ht, :, a0 : a0 + gw], ot[:])
                off_e += Ce
    nc.finalize()
    _prog_cache[key] = nc
    return nc


def _route(x: np.ndarray, w_gate: np.ndarray):
    """Replicates the reference router in fp32: softcapped softmax + top-2."""
    logits = x @ w_gate
    logits = (SOFTCAP * np.tanh(logits / SOFTCAP)).astype(np.float32)
    m = logits.max(axis=-1, keepdims=True)
    e = np.exp(logits - m)
    probs = e / e.sum(axis=-1, keepdims=True)
    idx = np.argsort(-probs, axis=-1, kind="stable")[:, :TOPK]
    return probs, idx


def _run(inputs, trace=False, trace_kwargs=None):
    hidden_states = np.asarray(inputs["hidden_states"], dtype=np.float32)
    w_gate = np.asarray(inputs["w_gate"], dtype=np.float32)
    w1 = np.asarray(inputs["w1"], dtype=np.float32)
    w3 = np.asarray(inputs["w3"], dtype=np.float32)
    w2 = np.asarray(inputs["w2"], dtype=np.float32)

    orig_shape = hidden_states.shape
    x = hidden_states.reshape(-1, H)
    T = x.shape[0]

    probs, idx = _route(x, w_gate)
    sel = np.zeros((T, E), dtype=bool)
    sel[np.arange(T), idx[:, 0]] = True
    sel[np.arange(T), idx[:, 1]] = True
    tok_idx = [np.nonzero(sel[:, e])[0] for e in range(E)]
    counts = [len(t) for t in tok_idx]
    offs = np.concatenate([[0], np.cumsum(counts)]).astype(np.int64)
    Ctot = int(offs[-1])

    nc = _build_program(counts)

    x_bf = x.astype(ml_dtypes.bfloat16)
    xg = np.empty((Ctot, H), dtype=ml_dtypes.bfloat16)
    for e in range(E):
        xg[offs[e] : offs[e + 1]] = x_bf[tok_idx[e]]
    # xT layout [128 p, KH k, Ctot c] with element [p,k,c] = xg[c, k*128+p]
    xT = np.ascontiguousarray(xg.T.reshape(KH, 128, Ctot).transpose(1, 0, 2))

    w1_bf = w1.astype(ml_dtypes.bfloat16)
    w3_bf = w3.astype(ml_dtypes.bfloat16)
    w2_bf = w2.astype(ml_dtypes.bfloat16)
    in_maps = []
    for c in range(8):
        sl = slice(c * ISL, (c + 1) * ISL)
        # [E, KL, 128(hsub), KH, 128(isub)]
        w1t = np.ascontiguousarray(
            w1_bf[:, :, sl].reshape(E, KH, 128, KL, 128).transpose(0, 3, 2, 1, 4)
        )
        w3t = np.ascontiguousarray(
            w3_bf[:, :, sl].reshape(E, KH, 128, KL, 128).transpose(0, 3, 2, 1, 4)
        )
        # [E, 128(isub), KH(ht), KL(k), 128(hsub)]
        w2t = np.ascontiguousarray(
            w2_bf[:, sl, :].reshape(E, KL, 128, KH, 128).transpose(0, 2, 3, 1, 4)
        )
        in_maps.append({"xT": xT, "w1t": w1t, "w3t": w3t, "w2t": w2t})

    res = run_bass_kernel_spmd(
        nc, in_maps, core_ids=list(range(8)), trace=trace,
        **(trace_kwargs or {}),
    )

    outT = np.zeros((H, Ctot), dtype=np.float32)
    for c in range(8):
        outT += res.results[c]["outT"].reshape(H, Ctot).astype(np.float32)

    out = np.zeros((T, H), dtype=np.float32)
    for e in range(E):
        wt = probs[tok_idx[e], e].astype(np.float32)
        out[tok_idx[e]] += outT[:, offs[e] : offs[e + 1]].T * wt[:, None]
    return out.reshape(orig_shape), res


def kernel(**inputs) -> np.ndarray:
    out, _ = _run(inputs, trace=False)
    return out
